# revision 8
# baseline (speedup 1.0000x reference)
"""AngularLayer Trainium2 kernel: [500000, 63] -> [500000, 483].

Per row: 21 (x,y) landmarks -> 210 ordered-pair unit direction vectors
(clipped x/y components), appended to the input row.

Sharded batch-parallel over 8 NeuronCores (62500 rows/core), SPMD one graph.
Layout per core: tiles of [125 partitions x R rows], features on the free
axis.  DVE does pair differences (interleaved (x,y) layout -- the only
fast DVE write/read order), ACT does squares + one dense rsqrt, GPSIMD
does clip+convert, DMA streams rows HBM<->SBUF.

Norm trick: instead of nsq[q] = sq[2q]+sq[2q+1] (strided, 2.3 cyc/elem on
DVE) followed by TWO strided rsqrt duplications on ACT, compute the
DUPLICATED-interleaved norm in one dense 2x DVE add:
    nsqd[2q+c] = sq[2q+c] + sq[2q + (1-c)]
where the second operand is the same 32-bit word with its two bf16 halves
swapped (inner step -1, still a packed aligned read).  One dense rsqrt on
nsqd then yields rrd already duplicated per component, so the tilt multiply
is a single fully-dense bf16 2x tensor_tensor.
"""

import os
from contextlib import ExitStack

import numpy as np

import concourse.bass as bass
import concourse.mybir as mybir
import concourse.tile as tile
from concourse import bacc
from concourse.bass_utils import run_bass_kernel_spmd

F32 = mybir.dt.float32
BF16 = mybir.dt.bfloat16
AF = mybir.ActivationFunctionType
ALU = mybir.AluOpType

N_CORES = 8
B_FULL = 500000
B_SHARD = B_FULL // N_CORES  # 62500
PARTS = 125
NLM = 21
NPAIR = 210
IN_C = 63
OUT_C = 483

ROWS_PER_PART = int(os.environ.get("ANGULAR_R", "10"))
SPLIT_A = int(os.environ.get("ANGULAR_SPLITA", "10"))  # subs i<SPLIT_A in phase A
GP_K = int(os.environ.get("ANGULAR_GPK", "16"))        # subs i>=GP_K on GPSIMD


def _build_nc(b_shard: int, rows_per_part: int) -> bass.Bass:
    R = rows_per_part
    assert b_shard % (PARTS * R) == 0
    n_tiles = b_shard // (PARTS * R)
    NF = R * 2 * NPAIR  # 4200 @ R=10: interleaved pair-component count

    nc = bacc.Bacc("TRN2", target_bir_lowering=False, debug=False)
    inp = nc.dram_tensor("tensor", [b_shard, IN_C], F32, kind="ExternalInput")
    outp = nc.dram_tensor("out", [b_shard, OUT_C], F32, kind="ExternalOutput")

    with tile.TileContext(nc) as tc, ExitStack() as ctx:
        opool = ctx.enter_context(tc.tile_pool(name="o", bufs=5))
        vpool = ctx.enter_context(tc.tile_pool(name="vxy", bufs=3))
        sqxp = ctx.enter_context(tc.tile_pool(name="sqx", bufs=2))
        npool = ctx.enter_context(tc.tile_pool(name="nsq", bufs=2))
        rrpool = ctx.enter_context(tc.tile_pool(name="rr", bufs=2))
        tpool = ctx.enter_context(tc.tile_pool(name="tt", bufs=2))

        st: dict = {}

        def _sub(t, i, eng):
            # one pair-difference: (x,y) of landmarks i+1..20 minus landmark i
            o3, vxy4 = st[t]["o3"], st[t]["vxy4"]
            np_i = NLM - 1 - i
            pb = i * (2 * NLM - 1 - i) // 2  # sum of (20-j) for j<i
            minu = o3[:, :, 3 * (i + 1):IN_C].rearrange(
                "p r (k three) -> p r k three", three=3)[:, :, :, 0:2]
            subt = o3[:, :, 3 * i:3 * i + 2].unsqueeze(2).broadcast_to(
                (PARTS, R, np_i, 2))
            eng.tensor_sub(vxy4[:, :, pb:pb + np_i, :], minu, subt)

        def stage_a1(t):
            # DMA in + first chunk of DVE subs + GPSIMD tail subs
            base = t * PARTS * R
            o = opool.tile([PARTS, R * OUT_C], F32, tag="o")
            o3 = o[:].rearrange("p (r c) -> p r c", c=OUT_C)

            src = inp[base:base + PARTS * R, :].rearrange(
                "(p r) c -> p r c", p=PARTS)
            nc.sync.dma_start(out=o3[0:60, :, 0:IN_C], in_=src[0:60].opt())
            nc.scalar.dma_start(out=o3[60:124, :, 0:IN_C], in_=src[60:124].opt())
            nc.sync.dma_start(out=o3[124:125, :, 0:IN_C], in_=src[124:125].opt())

            vxy = vpool.tile([PARTS, NF], BF16, tag="vxy")
            vxy4 = vxy[:].rearrange("p (r q two) -> p r q two", q=NPAIR, two=2)
            st[t] = {"o": o, "o3": o3, "vxy": vxy, "vxy4": vxy4}
            for i in range(SPLIT_A):
                _sub(t, i, nc.vector)
            for i in range(GP_K, NLM - 1):
                _sub(t, i, nc.gpsimd)

        def stage_a2(t):
            # second chunk of DVE subs (issued after add(t-1) so the ACT
            # rsqrt(t-1) round-trip hides under these)
            for i in range(SPLIT_A, GP_K):
                _sub(t, i, nc.vector)
            # squares can only run once all subs landed; queue after rsqrt(t-1)
            vxy = st[t]["vxy"]
            sq = sqxp.tile([PARTS, NF], BF16, tag="sqx")
            nc.scalar.activation(sq[:], vxy[:], AF.Square)
            st[t]["sq"] = sq

        def stage_b1(t):
            # duplicated norm (swap-add, dense 2x) -> one dense rsqrt
            sq = st[t].pop("sq")
            sqv = sq[:].rearrange("p (q two) -> p q two", two=2)
            nsqd = npool.tile([PARTS, NF], BF16, tag="nsq")
            nsqv = nsqd[:].rearrange("p (q two) -> p q two", two=2)
            nc.vector.tensor_add(nsqv, sqv, sqv[:, :, ::-1])

            rrd = rrpool.tile([PARTS, NF], BF16, tag="rr")
            nc.scalar.activation(rrd[:], nsqd[:], AF.Abs_reciprocal_sqrt)
            st[t]["rr"] = rrd

        def stage_b2(t):
            # dense 2x multiply
            vxy = st[t].pop("vxy")
            rrd = st[t].pop("rr")
            tt = tpool.tile([PARTS, NF], BF16, tag="tt")
            nc.vector.tensor_mul(tt[:], vxy[:], rrd[:])
            st[t]["tt"] = tt

        def stage_c(t):
            # clip + bf16->f32 [GPSIMD], DMA out
            base = t * PARTS * R
            o, o3, tt = (st[t][k] for k in ("o", "o3", "tt"))
            o_tilt = o3[:, :, IN_C:OUT_C]
            tt3 = tt[:].rearrange("p (r c) -> p r c", c=2 * NPAIR)
            nc.gpsimd.tensor_scalar(o_tilt, tt3, 1.0, -1.0, ALU.min, ALU.max)

            dst = outp[base:base + PARTS * R, :].rearrange(
                "(p r) c -> p (r c)", p=PARTS)
            nc.sync.dma_start(out=dst[0:60], in_=o[0:60, :])
            nc.scalar.dma_start(out=dst[60:124], in_=o[60:124, :])
            nc.sync.dma_start(out=dst[124:125], in_=o[124:125, :])
            del st[t]

        # DVE queue per iteration: subsA(s), add(s-1), subsB(s), mult(s-1);
        # the ACT rsqrt(s-1) runs while the DVE does subsB(s), so the
        # mult(s-1) no longer stalls on the same-tile rsqrt round-trip.
        for s in range(n_tiles + 2):
            if s >= 2:
                stage_c(s - 2)
            if s < n_tiles:
                stage_a1(s)
            if 1 <= s <= n_tiles:
                stage_b1(s - 1)
            if s < n_tiles:
                stage_a2(s)
            if 1 <= s <= n_tiles:
                stage_b2(s - 1)

    nc.compile()
    return nc


_NC_CACHE: dict = {}


def _get_nc():
    key = (B_SHARD, ROWS_PER_PART, SPLIT_A, GP_K)
    if key not in _NC_CACHE:
        _NC_CACHE[key] = _build_nc(B_SHARD, ROWS_PER_PART)
    return _NC_CACHE[key]


def kernel(tensor: np.ndarray) -> np.ndarray:
    tensor = np.ascontiguousarray(np.asarray(tensor, dtype=np.float32))
    assert tensor.shape == (B_FULL, IN_C), tensor.shape

    nc = _get_nc()
    in_maps = [
        {"tensor": tensor[c * B_SHARD:(c + 1) * B_SHARD]} for c in range(N_CORES)
    ]
    trace = os.environ.get("ANGULAR_TRACE", "0") == "1"
    res = run_bass_kernel_spmd(
        nc, in_maps, core_ids=list(range(N_CORES)), trace=trace
    )
    if trace:
        kernel.last_exec_time_ns = res.exec_time_ns
        kernel.last_results = res
    out = np.concatenate([res.results[c]["out"] for c in range(N_CORES)], axis=0)
    return out
